# revision 11
# baseline (speedup 1.0000x reference)
"""Trainium2 Bass kernel for MultiHeadedAttentionSANM (B=16, T=1024, F=512, H=4, K=11).

Sharding: data-parallel over batch across 8 NeuronCores (2 batch items per
core), no collectives. Host pre-transposes x to feature-major layout and
re-transposes the output; the mask is exploited as a valid-prefix (first
`nv` frames valid), detected on host.

Per-core dataflow (fp16 operands, fp32 PSUM accumulation everywhere):
  xT (feat x tok) --w_qkv--> qT,kT feature-major; v row-major; vmT feature-major
  scoresT[tk,tq] = kT_h.T @ qT_h   (keys on partitions; per key-tile the two
                                    512-wide halves land in one 2-bank PSUM
                                    pair, consumed by ONE 1024-wide Exp)
  expT = Exp(scale*scoresT - 3)    (ScalarE, PSUM->SBUF, fp16)
  es   = sum of the 6 exp tiles    (partial sums split DVE / GpSimd)
  dn   = ones.T @ es               (PE, M=1, into a freed scores pair)
  1/dn = Exp(-Ln(dn))              (ScalarE, [1,1024])
  bcs  = ones_col @ (1/dn)         (PE K=1 broadcast + copy to SBUF)
  ctxT_h = v_h.T @ expT            (PE accumulate, banks 4-5)
  ctx_norm = ctxT * bcs            (DVE mult, fused PSUM->SBUF fp16)
  fsmn: taps 0-3 on DVE (fp16 shifted scalar-mults on vTp); taps 4-10 +
        identity(residual) as 8 accumulating diagonal matmuls on the PE
        (banks 4-5 after ctx is consumed); combined into facc (SBUF f32)
  att_outT = w_out.T @ ctx_norm    (banks 6-7)
  out = att_outT + facc            (valid frames); att_outT on the padded tail

The two batch items are software-pipelined at instruction level: item1's
projection groups are emitted inside item0's attention head-slots (filling
the PE idle while ScalarE runs Exp), and item0's out-projection rides inside
item1's attention slots.
"""

import sys

sys.path.insert(0, "/opt/trn_rl_repo")

import numpy as np

import concourse.bass as bass
import concourse.mybir as mybir
import concourse.tile as tile
from concourse.bass_utils import run_bass_kernel_spmd

F32 = mybir.dt.float32
FP16 = mybir.dt.float16

N_CORES = 8
B, T, F = 16, 1024, 512
H, DK = 4, 128
FC = F // 128
KERNEL = 11
LEFT_PAD = (KERNEL - 1) // 2  # 5
NPE = 8  # PE fsmn passes: conv taps 4..10 plus identity (residual)
NDVE = 4  # DVE fsmn taps 0..3
NB = B // N_CORES
SCALE = DK ** -0.5
EXP_BIAS = -3.0  # constant shift inside exp; cancels in softmax normalization

Alu = mybir.AluOpType
Act = mybir.ActivationFunctionType


def _split_multiwaits(nc, max_waits=1):
    """walrus on this toolchain accepts at most one sync-wait command per
    instruction; split extras onto same-engine NoOps placed just before."""
    n_split = 0
    for fn in nc.m.functions:
        for bb in fn.blocks:
            out = []
            for inst in bb.instructions:
                si = inst.sync_info
                if si is not None and len(si.on_wait) > max_waits:
                    waits = list(si.on_wait)
                    for w in waits[:-max_waits]:
                        nop = mybir.InstNoOp(
                            name=nc.get_next_instruction_name(),
                            engine=inst.engine,
                            sync_info=mybir.SyncInfo(on_wait=[w], on_update=[]),
                            bass_nofuse=True,
                        )
                        out.append(nop)
                        n_split += 1
                    inst.sync_info = mybir.SyncInfo(
                        on_wait=waits[-max_waits:], on_update=list(si.on_update)
                    )
                out.append(inst)
            bb.instructions = out
    return n_split


def _ceil_div(a, b):
    return (a + b - 1) // b


def _n_chunks(n, c=512):
    out = []
    s = 0
    while s < n:
        out.append((s, min(c, n - s)))
        s += c
    return out


def _build(nv, use_bqkv, use_bout):
    nc = bass.Bass()

    nvt = _ceil_div(nv, 128)  # valid key tiles
    nv_part = nv - (nvt - 1) * 128  # rows in the last key tile
    TP = T + KERNEL - 1  # padded fsmn time extent

    xT_p = nc.declare_dram_parameter("xT", [NB, 128, FC, T], FP16, isOutput=False)
    wq_p = nc.declare_dram_parameter("wq", [128, FC, 3 * F], FP16, isOutput=False)
    wout_p = nc.declare_dram_parameter("wout", [128, FC, F], FP16, isOutput=False)
    wdiag_p = nc.declare_dram_parameter(
        "wdiag", [128, FC, NPE, 128], FP16, isOutput=False
    )
    wfsmn_p = nc.declare_dram_parameter("wfsmn", [128, FC, KERNEL], F32, isOutput=False)
    if use_bqkv:
        bqkv_p = nc.declare_dram_parameter("bqkv", [1, 3 * F], F32, isOutput=False)
    if use_bout:
        bout_p = nc.declare_dram_parameter("bout", [128, FC], F32, isOutput=False)
    if nv_part != 128:
        vcol_p = nc.declare_dram_parameter("vcol", [128, 1], F32, isOutput=False)
    out_p = nc.declare_dram_parameter("outT", [NB, F, T], FP16, isOutput=True)

    with tile.TileContext(nc) as tc:
        with (
            tc.tile_pool(name="consts", bufs=1) as consts,
            tc.tile_pool(name="xtr", bufs=2) as xtr,
            tc.tile_pool(name="peritem", bufs=2) as peritem,
            tc.tile_pool(name="expp", bufs=12) as expp,
            tc.tile_pool(name="esp", bufs=2) as esp,
            tc.tile_pool(name="bcsp", bufs=2) as bcsp,
            tc.tile_pool(name="smalls", bufs=2) as smalls,
            tc.tile_pool(name="f2p", bufs=2) as f2p,
            tc.tile_pool(name="accp", bufs=8) as accp,
            tc.tile_pool(name="finp", bufs=4) as finp,
            tc.tile_pool(name="psp", bufs=1, space="PSUM") as psp,
        ):
            # one flat PSUM tile = all 8 banks; manual bank layout:
            #   banks 0-1 / 2-3 : rotating scores pairs (+ dn / bcs bcast)
            #   banks 4-5       : ctx accumulate, then fsmn diag-matmuls
            #   banks 6-7       : projection / out-projection groups
            PS = psp.tile([128, 4096], F32, tag="PS")

            def pair(i):  # [128,1024] view of bank pair i (i in 0..3)
                return PS[:, i * 1024:(i + 1) * 1024]

            def bank(i):  # [128,512] view of bank i
                return PS[:, i * 512:(i + 1) * 512]

            # ---- DMAs for weights/inputs, finest-consumer-first order ----
            wq_t = consts.tile([128, FC, 3 * F], FP16, tag="wq")
            # q cols first (first thing the PE needs), then k, then v
            nc.sync.dma_start(out=wq_t[:, :, 0:F], in_=wq_p[:, :, 0:F])
            nc.sync.dma_start(out=wq_t[:, :, F:2 * F], in_=wq_p[:, :, F:2 * F])
            nc.sync.dma_start(out=wq_t[:, :, 2 * F:3 * F], in_=wq_p[:, :, 2 * F:3 * F])

            xT_t = [xtr.tile([128, FC, T], FP16, tag="xT", name=f"xT{i}")
                    for i in range(NB)]
            for item in range(NB):
                for ic in range(FC):
                    nc.scalar.dma_start(
                        out=xT_t[item][:, ic, :], in_=xT_p[item, :, ic, :]
                    )

            wout_e = consts.tile([128, FC, F], FP16, tag="wout")
            nc.sync.dma_start(out=wout_e, in_=wout_p[:, :, :])
            wfsmn_t = consts.tile([128, FC, KERNEL], F32, tag="wfsmn")
            nc.sync.dma_start(out=wfsmn_t, in_=wfsmn_p[:, :, :])
            wdiag = consts.tile([128, FC, NPE, 128], FP16, tag="wdiag")
            nc.scalar.dma_start(out=wdiag, in_=wdiag_p[:, :, :, :])

            # ---- small constants ----
            ones_col = consts.tile([128, 1], FP16, tag="onescol")
            tmp_oc = consts.tile([128, 1], F32, tag="onescol_f")
            nc.vector.memset(tmp_oc, 1.0)
            nc.vector.tensor_copy(ones_col, tmp_oc)
            ones_row = consts.tile([1, 128], FP16, tag="onesrow")
            tmp_or = consts.tile([1, 128], F32, tag="onesrow_f")
            nc.vector.memset(tmp_or, 1.0)
            nc.vector.tensor_copy(ones_row, tmp_or)
            expb = consts.tile([128, 1], F32, tag="expb")
            nc.vector.memset(expb, EXP_BIAS)
            if use_bqkv:
                ones_row512 = consts.tile([1, 512], FP16, tag="onesrow512")
                tmp_o5 = consts.tile([1, 512], F32, tag="onesrow512_f")
                nc.vector.memset(tmp_o5, 1.0)
                nc.vector.tensor_copy(ones_row512, tmp_o5)
                bqkv_stage = consts.tile([1, 3 * F], F32, tag="bqkv_f")
                nc.sync.dma_start(out=bqkv_stage, in_=bqkv_p[:, :])
                bqkv_a = consts.tile([1, 3 * F], FP16, tag="bqkv")
                nc.vector.tensor_copy(bqkv_a, bqkv_stage)
            if use_bout:
                bout_t = consts.tile([128, FC], F32, tag="bout")
                nc.sync.dma_start(out=bout_t, in_=bout_p[:, :])
            if nv_part != 128:
                vcol_t = consts.tile([128, 1], F32, tag="vcol")
                nc.sync.dma_start(out=vcol_t, in_=vcol_p[:, :])

            # ---- per-item persistent tiles ----
            qT = [peritem.tile([128, H, T], FP16, tag="qT", name=f"qT{i}")
                  for i in range(NB)]
            kT = [peritem.tile([128, H, nvt * 128], FP16, tag="kT", name=f"kT{i}")
                  for i in range(NB)]
            vrow = [peritem.tile([128, nvt, F], FP16, tag="vrow", name=f"vrow{i}")
                    for i in range(NB)]
            ctx = [peritem.tile([128, H, T], FP16, tag="ctx", name=f"ctx{i}")
                   for i in range(NB)]
            vTp = [peritem.tile([128, FC, TP], FP16, tag="vTp", name=f"vTp{i}")
                   for i in range(NB)]
            for item in range(NB):
                # zero only the pad slivers (left pad + masked tail)
                nc.vector.memset(vTp[item][:, :, 0:LEFT_PAD], 0.0)
                nc.vector.memset(vTp[item][:, :, LEFT_PAD + nv:TP], 0.0)

            faccs = [[None] * FC for _ in range(NB)]

            # rotating projection-bank pointer (banks 6,7) and scores-pair
            # pointer (pairs 0,1)
            state = {"pj": 0, "sp": 0}

            def next_pj_bank():
                b = 6 + state["pj"]
                state["pj"] ^= 1
                return b

            def next_spair():
                p = state["sp"]
                state["sp"] ^= 1
                return p

            def bias_mm(psum_ap, oc_global, nsz):
                nc.tensor.matmul(
                    psum_ap,
                    bqkv_a[:, oc_global * 128:(oc_global + 1) * 128],
                    ones_row512[:, 0:nsz],
                    start=False,
                    stop=True,
                )

            # ---- projection group emitters (closures for interleave) ----
            def fm_group(item, dst_fn, ocg, t0, tsz, copy_eng):
                """feature-major projection chunk; dst_fn(t0,tsz) -> dest AP"""
                def emit():
                    b = bank(next_pj_bank())
                    for ic in range(FC):
                        nc.tensor.matmul(
                            b[:, 0:tsz],
                            wq_t[:, ic, ocg * 128:(ocg + 1) * 128],
                            xT_t[item][:, ic, t0:t0 + tsz],
                            start=(ic == 0),
                            stop=(ic == FC - 1) and not use_bqkv,
                        )
                    if use_bqkv:
                        bias_mm(b[:, 0:tsz], ocg, tsz)
                    copy_eng(dst_fn(t0, tsz), b[:, 0:tsz])
                return emit

            def vrow_group(item, tt, copy_eng):
                def emit():
                    trows = min(128, nv - tt * 128)
                    b = bank(next_pj_bank())
                    for ic in range(FC):
                        nc.tensor.matmul(
                            b[:trows, :],
                            xT_t[item][:, ic, tt * 128:tt * 128 + trows],
                            wq_t[:, ic, 2 * F:3 * F],
                            start=(ic == 0),
                            stop=(ic == FC - 1) and not use_bqkv,
                        )
                    if use_bqkv:
                        nc.tensor.matmul(
                            b[:trows, :],
                            ones_row512[:, 0:trows],
                            bqkv_a[:, 2 * F:3 * F],
                            start=False,
                            stop=True,
                        )
                    copy_eng(vrow[item][:trows, tt, :], b[:trows, :])
                return emit

            def proj_groups(item, copy_eng):
                """all projection groups for one item, in dependency order"""
                def q_dst(h):
                    return lambda t0, tsz: qT[item][:, h, t0:t0 + tsz]

                def k_dst(h):
                    return lambda t0, tsz: kT[item][:, h, t0:t0 + tsz]

                def v_dst(cc):
                    return lambda t0, tsz: vTp[item][
                        :, cc, LEFT_PAD + t0:LEFT_PAD + t0 + tsz
                    ]

                gs = []
                for h in range(H):
                    for (t0, tsz) in _n_chunks(T):
                        gs.append(fm_group(item, q_dst(h), h, t0, tsz,
                                           copy_eng))
                for h in range(H):
                    for (t0, tsz) in _n_chunks(nv):
                        gs.append(fm_group(item, k_dst(h), FC + h,
                                           t0, tsz, copy_eng))
                for tt in range(nvt):
                    gs.append(vrow_group(item, tt, copy_eng))
                for cc in range(FC):
                    for (t0, tsz) in _n_chunks(nv):
                        gs.append(fm_group(item, v_dst(cc), 2 * FC + cc,
                                           t0, tsz, copy_eng))
                return gs

            def act_copy(dst, src):
                nc.scalar.copy(dst, src)

            def dve_copy(dst, src):
                nc.vector.tensor_copy(dst, src)

            # ---- attention slot machinery ----
            def emit_scores_kt(item, h, kt):
                """one key-tile of scores + its 1024-wide exp; returns et"""
                krows = min(128, nv - kt * 128)
                sp = pair(next_spair())
                for (q0, qsz) in _n_chunks(T):
                    nc.tensor.matmul(
                        sp[:krows, q0:q0 + qsz],
                        kT[item][:, h, kt * 128:kt * 128 + krows],
                        qT[item][:, h, q0:q0 + qsz],
                        start=True,
                        stop=True,
                    )
                et = expp.tile([128, T], FP16, tag="expT")
                nc.scalar.activation(
                    et[:krows, :], sp[:krows, :],
                    Act.Exp, bias=expb[:krows, 0:1], scale=SCALE,
                )
                if krows != 128:
                    nc.vector.tensor_scalar_mul(
                        et[:, :], et[:, :], vcol_t[:, 0:1]
                    )
                return et

            def emit_es_step(ets, esA, esB, kt):
                """incremental partial sums: first half DVE, second GpSimd"""
                nh = (nvt + 1) // 2
                et = ets[kt]
                if kt < nh:
                    if kt == 1:
                        esA = esp.tile([128, T], FP16, tag="esA")
                        nc.vector.tensor_tensor(
                            out=esA, in0=ets[0][:, :], in1=et[:, :], op=Alu.add
                        )
                    elif kt >= 2:
                        nc.vector.tensor_tensor(
                            out=esA, in0=esA, in1=et[:, :], op=Alu.add
                        )
                else:
                    if kt == nh + 1:
                        esB = esp.tile([128, T], FP16, tag="esB")
                        nc.gpsimd.tensor_tensor(
                            out=esB, in0=ets[nh][:, :], in1=et[:, :], op=Alu.add
                        )
                    elif kt >= nh + 2:
                        nc.gpsimd.tensor_tensor(
                            out=esB, in0=esB, in1=et[:, :], op=Alu.add
                        )
                return esA, esB

            def emit_es_final(ets, esA, esB):
                nh = (nvt + 1) // 2
                if esA is None:
                    esA = ets[0]
                if esB is None and nvt > nh:
                    esB = ets[nh]
                if esB is not None:
                    es = esp.tile([128, T], FP16, tag="es")
                    nc.vector.tensor_tensor(out=es, in0=esA, in1=esB,
                                            op=Alu.add)
                else:
                    es = esA
                return es

            def emit_dn(es):
                """denominator matmuls into a rotating scores pair"""
                dp = pair(next_spair())
                for (q0, qsz) in _n_chunks(T):
                    nc.tensor.matmul(
                        dp[0:1, q0:q0 + qsz],
                        ones_col[:, :],
                        es[:, q0:q0 + qsz],
                        start=True,
                        stop=True,
                    )
                return dp

            def emit_rec(dp):
                """1/dn on ScalarE from the dn rows of pair dp"""
                ln_dn = smalls.tile([1, T], F32, tag="ln_dn")
                nc.scalar.activation(ln_dn, dp[0:1, :], Act.Ln)
                rec_r = smalls.tile([1, T], FP16, tag="rec_r")
                nc.scalar.activation(rec_r, ln_dn[:, :], Act.Exp, scale=-1.0)
                return rec_r

            def emit_bcs(dp, rec_r, bcs_on_act):
                """K=1 broadcast matmul into dp, then copy to SBUF"""
                for (q0, qsz) in _n_chunks(T):
                    nc.tensor.matmul(
                        dp[:, q0:q0 + qsz],
                        ones_row[:, :],
                        rec_r[:, q0:q0 + qsz],
                        start=True,
                        stop=True,
                    )
                bcs = bcsp.tile([128, T], FP16, tag="bcs")
                if bcs_on_act:
                    nc.scalar.copy(bcs, dp[:, :])
                else:
                    nc.vector.tensor_copy(bcs, dp[:, :])
                return bcs

            def emit_ctx_fsmn(item, ph, ets, bcs):
                """ctx matmul + normalize + fsmn for head/chunk ph"""
                cp = pair(2)  # banks 4-5
                for kt in range(nvt):
                    krows = min(128, nv - kt * 128)
                    for (q0, qsz) in _n_chunks(T):
                        nc.tensor.matmul(
                            cp[:, q0:q0 + qsz],
                            vrow[item][:krows, kt, ph * 128:(ph + 1) * 128],
                            ets[kt][:krows, q0:q0 + qsz],
                            start=(kt == 0),
                            stop=(kt == nvt - 1),
                        )
                nc.vector.tensor_tensor(
                    out=ctx[item][:, ph, :], in0=cp[:, :], in1=bcs[:, :],
                    op=Alu.mult,
                )
                # fsmn chunk ph: DVE taps 0..3 into facc2 (fp16), PE taps
                # 4..10 + identity into banks 4-5 (after ctx consumed)
                cc = ph
                facc2 = f2p.tile([128, nv], FP16, tag="facc2")
                nc.vector.tensor_scalar_mul(
                    facc2, vTp[item][:, cc, 0:nv], wfsmn_t[:, cc, 0:1]
                )
                for j in range(1, NDVE):
                    nc.vector.scalar_tensor_tensor(
                        out=facc2,
                        in0=vTp[item][:, cc, j:j + nv],
                        scalar=wfsmn_t[:, cc, j:j + 1],
                        in1=facc2,
                        op0=Alu.mult,
                        op1=Alu.add,
                    )
                for j8 in range(NPE):
                    sh = (NDVE + j8) if j8 < NPE - 1 else LEFT_PAD
                    for (t0, tsz) in _n_chunks(nv):
                        nc.tensor.matmul(
                            cp[:, t0:t0 + tsz],
                            wdiag[:, cc, j8, :],
                            vTp[item][:, cc, t0 + sh:t0 + sh + tsz],
                            start=(j8 == 0),
                            stop=(j8 == NPE - 1),
                        )
                facc = accp.tile([128, nv], FP16, tag="facc")
                nc.vector.scalar_tensor_tensor(
                    out=facc,
                    in0=facc2,
                    scalar=(bout_t[:, cc:cc + 1] if use_bout else 0.0),
                    in1=cp[:, 0:nv],
                    op0=Alu.add,
                    op1=Alu.add,
                )
                faccs[item][cc] = facc

            def outproj_groups(item):
                """8 out-projection groups + fin combines + output DMA"""
                gs = []
                fins = {}

                def make_group(oc, t0, tsz, last):
                    def emit():
                        if oc not in fins:
                            fins[oc] = finp.tile([128, T], FP16, tag="fin",
                                                 name=f"fin{item}_{oc}")
                        fin = fins[oc]
                        b = bank(next_pj_bank())
                        for fc in range(FC):
                            nc.tensor.matmul(
                                b[:, 0:tsz],
                                wout_e[:, fc, oc * 128:(oc + 1) * 128],
                                ctx[item][:, fc, t0:t0 + tsz],
                                start=(fc == 0),
                                stop=(fc == FC - 1),
                            )
                        facc = faccs[item][oc]
                        if t0 < nv:
                            vsz = min(tsz, nv - t0)
                            nc.vector.scalar_tensor_tensor(
                                out=fin[:, t0:t0 + vsz],
                                in0=facc[:, t0:t0 + vsz],
                                scalar=1.0,
                                in1=b[:, 0:vsz],
                                op0=Alu.bypass,
                                op1=Alu.add,
                            )
                        if t0 + tsz > nv:
                            p0 = max(t0, nv)
                            if use_bout:
                                nc.scalar.activation(
                                    fin[:, p0:t0 + tsz],
                                    b[:, p0 - t0:tsz],
                                    Act.Copy,
                                    bias=bout_t[:, oc:oc + 1],
                                )
                            else:
                                nc.scalar.copy(
                                    fin[:, p0:t0 + tsz], b[:, p0 - t0:tsz]
                                )
                        if last:
                            dma = nc.sync.dma_start if oc % 2 == 0 \
                                else nc.scalar.dma_start
                            dma(
                                out=out_p[item, oc * 128:(oc + 1) * 128, :],
                                in_=fin,
                            )
                    return emit

                for oc in range(FC):
                    chunks = _n_chunks(T)
                    for i, (t0, tsz) in enumerate(chunks):
                        gs.append(make_group(oc, t0, tsz, i == len(chunks) - 1))
                return gs

            def emit_attention(item, interleave, per_slot):
                """head-pipelined attention; drains `interleave` closures
                inside the slots (per_slot per head slot).

                Slot h: dn/rec/bcs of head h-1 first (their inputs finished
                last slot, so the rec chain beats this slot's exps into the
                ScalarE queue), then scores/exp/es of head h with the
                previous head's ctx+fsmn and interleave work spread in."""
                pend = list(interleave)

                def mk_drain(budget):
                    state = [budget]

                    def drain(k):
                        n = min(k, state[0], len(pend))
                        state[0] -= n
                        for _ in range(n):
                            pend.pop(0)()
                    return drain

                prev = None  # (h, ets, es)
                for step in range(H + 1):
                    drain = mk_drain(per_slot if step < H else len(pend))
                    dp = rec_r = None
                    if prev is not None:
                        ph, pets, pes = prev
                        dp = emit_dn(pes)
                        rec_r = emit_rec(dp)
                    if step < H:
                        h = step
                        ets = []
                        esA = esB = None
                        for kt in range(nvt):
                            ets.append(emit_scores_kt(item, h, kt))
                            esA, esB = emit_es_step(ets, esA, esB, kt)
                            if kt == 1 and dp is not None:
                                bcs = emit_bcs(dp, rec_r,
                                               bcs_on_act=(step % 2 == 1))
                            drain(1)
                        es = emit_es_final(ets, esA, esB)
                        drain(1)
                        if prev is not None:
                            emit_ctx_fsmn(item, ph, pets, bcs)
                        drain(per_slot)
                        prev = (h, ets, es)
                    else:
                        bcs = emit_bcs(dp, rec_r, bcs_on_act=(step % 2 == 1))
                        emit_ctx_fsmn(item, prev[0], prev[1], bcs)
                        drain(len(pend))
                return pend

            # =========================== schedule ===========================
            # P0: item0 projections (PE-dense; copies on ACT which is idle)
            for g in proj_groups(0, act_copy):
                g()
            # A0: item0 attention with item1 projection groups interleaved
            rest = emit_attention(0, proj_groups(1, dve_copy), per_slot=8)
            for g in rest:
                g()
            # A1: item1 attention with item0 out-projection interleaved
            rest = emit_attention(1, outproj_groups(0), per_slot=2)
            for g in rest:
                g()
            # O1: item1 out-projection
            for g in outproj_groups(1):
                g()

    _split_multiwaits(nc)
    return nc


_cache = {}


def _get_nc(nv, use_bqkv, use_bout):
    key = (nv, use_bqkv, use_bout)
    if key not in _cache:
        _cache[key] = _build(nv, use_bqkv, use_bout)
    return _cache[key]


def _make_wdiag(w_fsmn):
    """(128, FC, NPE, 128) fp16: diag(w[:, 4..10]) + identity (residual)."""
    wd = np.zeros((128, FC, NPE, 128), np.float16)
    idx = np.arange(128)
    for cc in range(FC):
        for j8 in range(NPE - 1):
            wd[idx, cc, j8, idx] = w_fsmn[cc * 128 + idx, NDVE + j8].astype(
                np.float16
            )
        wd[idx, cc, NPE - 1, idx] = 1.0
    return wd


def kernel(x, mask, w_qkv, b_qkv, w_out, b_out, w_fsmn):
    x = np.asarray(x, dtype=np.float32)
    mask = np.asarray(mask, dtype=np.float32)
    w_qkv = np.asarray(w_qkv, dtype=np.float32)
    b_qkv = np.asarray(b_qkv, dtype=np.float32)
    w_out = np.asarray(w_out, dtype=np.float32)
    b_out = np.asarray(b_out, dtype=np.float32)
    w_fsmn = np.asarray(w_fsmn, dtype=np.float32)

    assert x.shape == (B, T, F) and mask.shape == (B, 1, T)

    # mask must be a shared valid-prefix across the batch (as in batched ASR)
    m = mask.reshape(B, T)
    nv = int(round(float(m[0].sum())))
    expect = np.zeros(T, np.float32)
    expect[:nv] = 1.0
    if not np.all(m == expect[None, :]):
        raise NotImplementedError("kernel supports shared prefix masks only")
    nv = max(128, min(T, nv))

    use_bqkv = bool(np.any(b_qkv))
    use_bout = bool(np.any(b_out))
    nc = _get_nc(nv, use_bqkv, use_bout)

    nvt = _ceil_div(nv, 128)
    wdiag = _make_wdiag(w_fsmn)
    wfsmn_t = np.ascontiguousarray(
        w_fsmn.reshape(FC, 128, KERNEL).transpose(1, 0, 2)
    )
    # xT as [NB, 128, FC, T]: xT[i, p, c, t] = x[i, t, c*128+p]
    x16 = x.astype(np.float16)
    xT16 = [
        np.ascontiguousarray(
            x16[c * NB:(c + 1) * NB]
            .transpose(0, 2, 1)                     # (NB, F, T)
            .reshape(NB, FC, 128, T)
            .transpose(0, 2, 1, 3)                  # (NB, 128, FC, T)
        )
        for c in range(N_CORES)
    ]
    # wq as [128, FC, 3F]: wq[p, c, o] = w_qkv[c*128+p, o]
    wq16 = np.ascontiguousarray(
        w_qkv.astype(np.float16).reshape(FC, 128, 3 * F).transpose(1, 0, 2)
    )
    # wout as [128, FC, F]
    wout16 = np.ascontiguousarray(
        w_out.astype(np.float16).reshape(FC, 128, F).transpose(1, 0, 2)
    )
    in_maps = []
    for c in range(N_CORES):
        im = {
            "xT": xT16[c],
            "wq": wq16,
            "wout": wout16,
            "wdiag": wdiag,
            "wfsmn": wfsmn_t,
        }
        if use_bqkv:
            im["bqkv"] = np.ascontiguousarray(b_qkv[None, :])
        if use_bout:
            im["bout"] = np.ascontiguousarray(b_out.reshape(FC, 128).T)
        if nv - (nvt - 1) * 128 != 128:
            vcol = np.zeros((128, 1), np.float32)
            vcol[: nv - (nvt - 1) * 128] = 1.0
            im["vcol"] = vcol
        in_maps.append(im)

    global _last_in_maps
    _last_in_maps = in_maps
    res = run_bass_kernel_spmd(nc, in_maps, list(range(N_CORES)))
    out = np.empty((B, T, F), np.float32)
    for c in range(N_CORES):
        oT = res.results[c]["outT"]  # (NB, F, T) fp16
        for i in range(NB):
            out[c * NB + i] = oT[i].astype(np.float32).T
    return out


# revision 24
# speedup vs baseline: 1.1396x; 1.1396x over previous
"""Trainium2 Bass kernel for MultiHeadedAttentionSANM (B=16, T=1024, F=512, H=4, K=11).

Sharding: data-parallel over batch across 8 NeuronCores (2 batch items per
core), no collectives. Host pre-transposes x to feature-major layout and
re-transposes the output; the mask is exploited as a valid-prefix (first
`nv` frames valid), detected on host.

Per-core dataflow (fp16 operands, fp32 PSUM accumulation everywhere):
  xT (feat x tok) --w_qkv--> qT,kT feature-major; v row-major; vmT feature-major
  scoresT[tk,tq] = kT_h.T @ qT_h   (keys on partitions; per key-tile the two
                                    512-wide halves land in one 2-bank PSUM
                                    pair, consumed by ONE 1024-wide Exp)
  expT = Exp(scale*scoresT - 3)    (ScalarE, PSUM->SBUF, fp16)
  es   = sum of the 6 exp tiles    (partial sums split DVE / GpSimd)
  dn   = ones.T @ es               (PE, M=1, into a freed scores pair)
  1/dn = Exp(-Ln(dn))              (ScalarE, [1,1024])
  bcs  = ones_col @ (1/dn)         (PE K=1 broadcast + copy to SBUF)
  ctxT_h = v_h.T @ expT            (PE accumulate, banks 4-5)
  ctx_norm = ctxT * bcs            (DVE mult, fused PSUM->SBUF fp16)
  fsmn: taps 0-3 on DVE (fp16 shifted scalar-mults on vTp); taps 4-10 +
        identity(residual) as 8 accumulating diagonal matmuls on the PE
        (banks 4-5 after ctx is consumed); combined into facc (SBUF f32)
  att_outT = w_out.T @ ctx_norm    (banks 6-7)
  out = att_outT + facc            (valid frames); att_outT on the padded tail

The two batch items are software-pipelined at instruction level: item1's
projection groups are emitted inside item0's attention head-slots (filling
the PE idle while ScalarE runs Exp), and item0's out-projection rides inside
item1's attention slots.
"""

import sys

sys.path.insert(0, "/opt/trn_rl_repo")

import numpy as np

import concourse.bass as bass
import concourse.mybir as mybir
import concourse.tile as tile
from concourse.bass_utils import run_bass_kernel_spmd

F32 = mybir.dt.float32
FP16 = mybir.dt.float16

N_CORES = 8
B, T, F = 16, 1024, 512
H, DK = 4, 128
FC = F // 128
KERNEL = 11
LEFT_PAD = (KERNEL - 1) // 2  # 5
NPE = 8  # PE fsmn passes: conv taps 4..10 plus identity (residual)
NDVE = 4  # DVE fsmn taps 0..3
NB = B // N_CORES
SCALE = DK ** -0.5
EXP_BIAS = -3.0  # constant shift inside exp; cancels in softmax normalization

Alu = mybir.AluOpType
Act = mybir.ActivationFunctionType


def _split_multiwaits(nc, max_waits=1):
    """walrus on this toolchain accepts at most one sync-wait command per
    instruction; split extras onto same-engine NoOps placed just before."""
    n_split = 0
    for fn in nc.m.functions:
        for bb in fn.blocks:
            out = []
            for inst in bb.instructions:
                si = inst.sync_info
                if si is not None and len(si.on_wait) > max_waits:
                    waits = list(si.on_wait)
                    for w in waits[:-max_waits]:
                        nop = mybir.InstNoOp(
                            name=nc.get_next_instruction_name(),
                            engine=inst.engine,
                            sync_info=mybir.SyncInfo(on_wait=[w], on_update=[]),
                            bass_nofuse=True,
                        )
                        out.append(nop)
                        n_split += 1
                    inst.sync_info = mybir.SyncInfo(
                        on_wait=waits[-max_waits:], on_update=list(si.on_update)
                    )
                out.append(inst)
            bb.instructions = out
    return n_split


def _ceil_div(a, b):
    return (a + b - 1) // b


def _n_chunks(n, c=512):
    out = []
    s = 0
    while s < n:
        out.append((s, min(c, n - s)))
        s += c
    return out


def _build(nv, use_bqkv, use_bout):
    nc = bass.Bass()

    nvt = _ceil_div(nv, 128)  # valid key tiles
    nv_part = nv - (nvt - 1) * 128  # rows in the last key tile
    TP = T + KERNEL - 1  # padded fsmn time extent

    xT_p = nc.declare_dram_parameter("xT", [NB, 128, FC, T], FP16, isOutput=False)
    wq_p = nc.declare_dram_parameter("wq", [128, FC, 3 * F], FP16, isOutput=False)
    wout_p = nc.declare_dram_parameter("wout", [128, FC, F], FP16, isOutput=False)
    wdiag_p = nc.declare_dram_parameter(
        "wdiag", [128, FC, NPE, 128], FP16, isOutput=False
    )
    wfsmn_p = nc.declare_dram_parameter("wfsmn", [128, FC, KERNEL], F32, isOutput=False)
    if use_bqkv:
        bqkv_p = nc.declare_dram_parameter("bqkv", [1, 3 * F], F32, isOutput=False)
    if use_bout:
        bout_p = nc.declare_dram_parameter("bout", [128, FC], F32, isOutput=False)
    if nv_part != 128:
        vcol_p = nc.declare_dram_parameter("vcol", [128, 1], F32, isOutput=False)
    out_p = nc.declare_dram_parameter("outT", [NB, F, T], FP16, isOutput=True)

    with tile.TileContext(nc) as tc:
        with (
            tc.tile_pool(name="consts", bufs=1) as consts,
            tc.tile_pool(name="xtr", bufs=2) as xtr,
            tc.tile_pool(name="peritem", bufs=2) as peritem,
            tc.tile_pool(name="expp", bufs=12) as expp,
            tc.tile_pool(name="esp", bufs=2) as esp,
            tc.tile_pool(name="bcsp", bufs=2) as bcsp,
            tc.tile_pool(name="smalls", bufs=2) as smalls,
            tc.tile_pool(name="f2p", bufs=2) as f2p,
            tc.tile_pool(name="accp", bufs=8) as accp,
            tc.tile_pool(name="finp", bufs=4) as finp,
            tc.tile_pool(name="psp", bufs=1, space="PSUM") as psp,
        ):
            # one flat PSUM tile = all 8 banks; manual bank layout:
            #   banks 0-1 / 2-3 : rotating scores pairs (+ dn / bcs bcast)
            #   banks 4-5       : ctx accumulate, then fsmn diag-matmuls
            #   banks 6-7       : projection / out-projection groups
            PS = psp.tile([128, 4096], F32, tag="PS")

            def pair(i):  # [128,1024] view of bank pair i (i in 0..3)
                return PS[:, i * 1024:(i + 1) * 1024]

            def bank(i):  # [128,512] view of bank i
                return PS[:, i * 512:(i + 1) * 512]

            # ---- DMAs for weights/inputs, finest-consumer-first order ----
            wq_t = consts.tile([128, FC, 3 * F], FP16, tag="wq")
            # q cols first (first thing the PE needs), then k, then v;
            # split per input-chunk so the first accumulation group can
            # start as soon as chunk 0 lands
            for sec in range(3):
                for ic in range(FC):
                    nc.sync.dma_start(
                        out=wq_t[:, ic, sec * F:(sec + 1) * F],
                        in_=wq_p[:, ic, sec * F:(sec + 1) * F],
                    )

            xT_t = [xtr.tile([128, FC, T], FP16, tag="xT", name=f"xT{i}")
                    for i in range(NB)]
            for item in range(NB):
                for ic in range(FC):
                    nc.scalar.dma_start(
                        out=xT_t[item][:, ic, :], in_=xT_p[item, :, ic, :]
                    )

            wout_e = consts.tile([128, FC, F], FP16, tag="wout")
            nc.sync.dma_start(out=wout_e, in_=wout_p[:, :, :])
            wfsmn_t = consts.tile([128, FC, KERNEL], F32, tag="wfsmn")
            nc.sync.dma_start(out=wfsmn_t, in_=wfsmn_p[:, :, :])
            wdiag = consts.tile([128, FC, NPE, 128], FP16, tag="wdiag")
            nc.scalar.dma_start(out=wdiag, in_=wdiag_p[:, :, :, :])

            # ---- small constants ----
            ones_col = consts.tile([128, 1], FP16, tag="onescol")
            tmp_oc = consts.tile([128, 1], F32, tag="onescol_f")
            nc.vector.memset(tmp_oc, 1.0)
            nc.vector.tensor_copy(ones_col, tmp_oc)
            ones_row = consts.tile([64, 128], FP16, tag="onesrow")
            tmp_or = consts.tile([64, 128], F32, tag="onesrow_f")
            nc.vector.memset(tmp_or, 1.0)
            nc.vector.tensor_copy(ones_row, tmp_or)
            expb = consts.tile([128, 1], F32, tag="expb")
            nc.vector.memset(expb, EXP_BIAS)
            if use_bqkv:
                ones_row512 = consts.tile([1, 512], FP16, tag="onesrow512")
                tmp_o5 = consts.tile([1, 512], F32, tag="onesrow512_f")
                nc.vector.memset(tmp_o5, 1.0)
                nc.vector.tensor_copy(ones_row512, tmp_o5)
                bqkv_stage = consts.tile([1, 3 * F], F32, tag="bqkv_f")
                nc.sync.dma_start(out=bqkv_stage, in_=bqkv_p[:, :])
                bqkv_a = consts.tile([1, 3 * F], FP16, tag="bqkv")
                nc.vector.tensor_copy(bqkv_a, bqkv_stage)
            if use_bout:
                bout_t = consts.tile([128, FC], F32, tag="bout")
                nc.sync.dma_start(out=bout_t, in_=bout_p[:, :])
            if nv_part != 128:
                vcol_t = consts.tile([128, 1], F32, tag="vcol")
                nc.sync.dma_start(out=vcol_t, in_=vcol_p[:, :])

            # ---- per-item persistent tiles ----
            qT = [peritem.tile([128, H, T], FP16, tag="qT", name=f"qT{i}")
                  for i in range(NB)]
            kT = [peritem.tile([128, H, nvt * 128], FP16, tag="kT", name=f"kT{i}")
                  for i in range(NB)]
            vrow = [peritem.tile([128, nvt, F], FP16, tag="vrow", name=f"vrow{i}")
                    for i in range(NB)]
            ctx = [peritem.tile([128, H, T], FP16, tag="ctx", name=f"ctx{i}")
                   for i in range(NB)]
            vTp = [peritem.tile([128, FC, TP], FP16, tag="vTp", name=f"vTp{i}")
                   for i in range(NB)]
            for item in range(NB):
                # zero only the pad slivers (left pad + masked tail)
                nc.vector.memset(vTp[item][:, :, 0:LEFT_PAD], 0.0)
                nc.vector.memset(vTp[item][:, :, LEFT_PAD + nv:TP], 0.0)

            faccs = [[None] * FC for _ in range(NB)]

            # bank map during attention:
            #   pairs 0,1 (banks 0-3): rotating scores tiles
            #   bank 4 rows {0,32}   : softmax denominator (+ proj rotation)
            #   banks 4,5            : projection-group rotation
            #   banks 6,7            : bps -> ctx -> fsmn chain
            state = {"pj": 0, "sp": 0}

            def next_pj_bank():
                b = 4 + state["pj"]
                state["pj"] ^= 1
                return b

            def next_spair():
                p = state["sp"]
                state["sp"] ^= 1
                return p

            def bias_mm(psum_ap, oc_global, nsz):
                nc.tensor.matmul(
                    psum_ap,
                    bqkv_a[:, oc_global * 128:(oc_global + 1) * 128],
                    ones_row512[:, 0:nsz],
                    start=False,
                    stop=True,
                )

            # ---- projection group emitters (closures for interleave) ----
            def fm_group(item, dst_fn, ocg, t0, tsz, copy_eng):
                """feature-major projection chunk; dst_fn(t0,tsz) -> dest AP"""
                def emit():
                    b = bank(next_pj_bank())
                    for ic in range(FC):
                        nc.tensor.matmul(
                            b[:, 0:tsz],
                            wq_t[:, ic, ocg * 128:(ocg + 1) * 128],
                            xT_t[item][:, ic, t0:t0 + tsz],
                            start=(ic == 0),
                            stop=(ic == FC - 1) and not use_bqkv,
                        )
                    if use_bqkv:
                        bias_mm(b[:, 0:tsz], ocg, tsz)
                    copy_eng(dst_fn(t0, tsz), b[:, 0:tsz])
                return emit

            def vrow_group(item, tt, copy_eng):
                def emit():
                    trows = min(128, nv - tt * 128)
                    b = bank(next_pj_bank())
                    for ic in range(FC):
                        nc.tensor.matmul(
                            b[:trows, :],
                            xT_t[item][:, ic, tt * 128:tt * 128 + trows],
                            wq_t[:, ic, 2 * F:3 * F],
                            start=(ic == 0),
                            stop=(ic == FC - 1) and not use_bqkv,
                        )
                    if use_bqkv:
                        nc.tensor.matmul(
                            b[:trows, :],
                            ones_row512[:, 0:trows],
                            bqkv_a[:, 2 * F:3 * F],
                            start=False,
                            stop=True,
                        )
                    copy_eng(vrow[item][:trows, tt, :], b[:trows, :])
                return emit

            def proj_groups(item, copy_eng):
                """all projection groups for one item, in dependency order"""
                def q_dst(h):
                    return lambda t0, tsz: qT[item][:, h, t0:t0 + tsz]

                def k_dst(h):
                    return lambda t0, tsz: kT[item][:, h, t0:t0 + tsz]

                def v_dst(cc):
                    return lambda t0, tsz: vTp[item][
                        :, cc, LEFT_PAD + t0:LEFT_PAD + t0 + tsz
                    ]

                gs = []
                for h in range(H):
                    for (t0, tsz) in _n_chunks(T):
                        gs.append(fm_group(item, q_dst(h), h, t0, tsz,
                                           copy_eng))
                for h in range(H):
                    for (t0, tsz) in _n_chunks(nv):
                        gs.append(fm_group(item, k_dst(h), FC + h,
                                           t0, tsz, copy_eng))
                for tt in range(nvt):
                    gs.append(vrow_group(item, tt, copy_eng))
                for cc in range(FC):
                    for (t0, tsz) in _n_chunks(nv):
                        gs.append(fm_group(item, v_dst(cc), 2 * FC + cc,
                                           t0, tsz, copy_eng))
                return gs

            def act_copy(dst, src):
                nc.scalar.copy(dst, src)

            def dve_copy(dst, src):
                nc.vector.tensor_copy(dst, src)

            def emit_proj_waves(item, copy_eng):
                """startup projections: 8 accumulation groups in flight
                across all 8 banks, input-chunk-outer so compute pipelines
                with the xT/wq DMA arrival"""
                def q_dst(h):
                    return lambda t0, tsz: qT[item][:, h, t0:t0 + tsz]

                def k_dst(h):
                    return lambda t0, tsz: kT[item][:, h, t0:t0 + tsz]

                def v_dst(cc):
                    return lambda t0, tsz: vTp[item][
                        :, cc, LEFT_PAD + t0:LEFT_PAD + t0 + tsz
                    ]

                def fm_wave(specs):
                    # specs: list of (ocg, t0, tsz, dst_fn), at most 8
                    for ic in range(FC):
                        for g, (ocg, t0, tsz, _) in enumerate(specs):
                            nc.tensor.matmul(
                                bank(g)[:, 0:tsz],
                                wq_t[:, ic, ocg * 128:(ocg + 1) * 128],
                                xT_t[item][:, ic, t0:t0 + tsz],
                                start=(ic == 0),
                                stop=(ic == FC - 1) and not use_bqkv,
                            )
                    for g, (ocg, t0, tsz, dst_fn) in enumerate(specs):
                        if use_bqkv:
                            bias_mm(bank(g)[:, 0:tsz], ocg, tsz)
                        copy_eng(dst_fn(t0, tsz), bank(g)[:, 0:tsz])

                fm_wave([(h, t0, tsz, q_dst(h))
                         for h in range(H) for (t0, tsz) in _n_chunks(T)])
                fm_wave([(FC + h, t0, tsz, k_dst(h))
                         for h in range(H) for (t0, tsz) in _n_chunks(nv)])
                # v row-major wave
                for ic in range(FC):
                    for tt in range(nvt):
                        trows = min(128, nv - tt * 128)
                        nc.tensor.matmul(
                            bank(tt)[:trows, :],
                            xT_t[item][:, ic, tt * 128:tt * 128 + trows],
                            wq_t[:, ic, 2 * F:3 * F],
                            start=(ic == 0),
                            stop=(ic == FC - 1) and not use_bqkv,
                        )
                for tt in range(nvt):
                    trows = min(128, nv - tt * 128)
                    if use_bqkv:
                        nc.tensor.matmul(
                            bank(tt)[:trows, :],
                            ones_row512[:, 0:trows],
                            bqkv_a[:, 2 * F:3 * F],
                            start=False,
                            stop=True,
                        )
                    copy_eng(vrow[item][:trows, tt, :], bank(tt)[:trows, :])
                fm_wave([(2 * FC + cc, t0, tsz, v_dst(cc))
                         for cc in range(FC) for (t0, tsz) in _n_chunks(nv)])

            # ---- attention slot machinery ----
            def emit_scores_kt(item, h, kt):
                """one key-tile of scores + its 1024-wide exp; returns et"""
                krows = min(128, nv - kt * 128)
                sp = pair(next_spair())
                for (q0, qsz) in _n_chunks(T):
                    nc.tensor.matmul(
                        sp[:krows, q0:q0 + qsz],
                        kT[item][:, h, kt * 128:kt * 128 + krows],
                        qT[item][:, h, q0:q0 + qsz],
                        start=True,
                        stop=True,
                    )
                et = expp.tile([128, T], FP16, tag="expT")
                nc.scalar.activation(
                    et[:krows, :], sp[:krows, :],
                    Act.Exp, bias=expb[:krows, 0:1], scale=SCALE,
                )
                if krows != 128:
                    nc.vector.tensor_scalar_mul(
                        et[:, :], et[:, :], vcol_t[:, 0:1]
                    )
                return et

            def emit_es_step(ets, es, kt):
                """incremental exp-tile sum on DVE"""
                et = ets[kt]
                if kt == 1:
                    es = esp.tile([128, T], FP16, tag="es")
                    nc.vector.tensor_tensor(
                        out=es, in0=ets[0][:, :], in1=et[:, :], op=Alu.add
                    )
                elif kt >= 2:
                    nc.vector.tensor_tensor(
                        out=es, in0=es, in1=et[:, :], op=Alu.add
                    )
                return es

            def emit_dn(es):
                """denominator matmuls into row 0 of banks 4-5"""
                for (q0, qsz) in _n_chunks(T):
                    nc.tensor.matmul(
                        PS[0:1, 2048 + q0:2048 + q0 + qsz],
                        ones_col[:, :],
                        es[:, q0:q0 + qsz],
                        start=True,
                        stop=True,
                    )

            def emit_rec():
                """1/dn on ScalarE from row 0 of banks 4-5"""
                ln_dn = smalls.tile([1, T], F32, tag="ln_dn")
                nc.scalar.activation(ln_dn, PS[0:1, 2048:2048 + T], Act.Ln)
                rec_r = smalls.tile([1, T], FP16, tag="rec_r")
                nc.scalar.activation(rec_r, ln_dn[:, :], Act.Exp, scale=-1.0)
                return rec_r

            def emit_bcs(rec_r):
                """K=1 broadcast matmul into banks 4-5, then DVE copy out"""
                bp = pair(2)
                for (q0, qsz) in _n_chunks(T):
                    nc.tensor.matmul(
                        bp[:, q0:q0 + qsz],
                        ones_row[0:1, :],
                        rec_r[0:1, q0:q0 + qsz],
                        start=True,
                        stop=True,
                    )
                bcs = bcsp.tile([128, T], FP16, tag="bcs")
                nc.vector.tensor_copy(bcs, bp[:, :])
                return bcs

            def emit_ctx(item, ph, ets):
                """ctx matmul for head ph into banks 6-7"""
                cp = pair(3)
                for kt in range(nvt):
                    krows = min(128, nv - kt * 128)
                    for (q0, qsz) in _n_chunks(T):
                        nc.tensor.matmul(
                            cp[:, q0:q0 + qsz],
                            vrow[item][:krows, kt, ph * 128:(ph + 1) * 128],
                            ets[kt][:krows, q0:q0 + qsz],
                            start=(kt == 0),
                            stop=(kt == nvt - 1),
                        )

            def emit_norm_fsmn(item, ph, bcs):
                """normalize ctx, then fsmn diag-matmuls reuse banks 6-7"""
                cp = pair(3)
                nc.vector.tensor_tensor(
                    out=ctx[item][:, ph, :], in0=cp[:, :], in1=bcs[:, :],
                    op=Alu.mult,
                )
                # fsmn chunk ph: DVE taps 0..3 into facc2 (fp16), PE taps
                # 4..10 + identity into banks 6-7 (after ctx consumed)
                cc = ph
                facc2 = f2p.tile([128, nv], FP16, tag="facc2")
                nc.vector.tensor_scalar_mul(
                    facc2, vTp[item][:, cc, 0:nv], wfsmn_t[:, cc, 0:1]
                )
                for j in range(1, NDVE):
                    nc.vector.scalar_tensor_tensor(
                        out=facc2,
                        in0=vTp[item][:, cc, j:j + nv],
                        scalar=wfsmn_t[:, cc, j:j + 1],
                        in1=facc2,
                        op0=Alu.mult,
                        op1=Alu.add,
                    )
                for j8 in range(NPE):
                    sh = (NDVE + j8) if j8 < NPE - 1 else LEFT_PAD
                    for (t0, tsz) in _n_chunks(nv):
                        nc.tensor.matmul(
                            cp[:, t0:t0 + tsz],
                            wdiag[:, cc, j8, :],
                            vTp[item][:, cc, t0 + sh:t0 + sh + tsz],
                            start=(j8 == 0),
                            stop=(j8 == NPE - 1),
                        )
                facc = accp.tile([128, nv], FP16, tag="facc")
                nc.vector.scalar_tensor_tensor(
                    out=facc,
                    in0=facc2,
                    scalar=(bout_t[:, cc:cc + 1] if use_bout else 0.0),
                    in1=cp[:, 0:nv],
                    op0=Alu.add,
                    op1=Alu.add,
                )
                faccs[item][cc] = facc

            def outproj_groups(item):
                """8 out-projection groups + fin combines + output DMA"""
                gs = []
                fins = {}

                def make_group(oc, t0, tsz, last):
                    def emit():
                        if oc not in fins:
                            fins[oc] = finp.tile([128, T], FP16, tag="fin",
                                                 name=f"fin{item}_{oc}")
                        fin = fins[oc]
                        b = bank(next_pj_bank())
                        for fc in range(FC):
                            nc.tensor.matmul(
                                b[:, 0:tsz],
                                wout_e[:, fc, oc * 128:(oc + 1) * 128],
                                ctx[item][:, fc, t0:t0 + tsz],
                                start=(fc == 0),
                                stop=(fc == FC - 1),
                            )
                        facc = faccs[item][oc]
                        if t0 < nv:
                            vsz = min(tsz, nv - t0)
                            nc.vector.scalar_tensor_tensor(
                                out=fin[:, t0:t0 + vsz],
                                in0=facc[:, t0:t0 + vsz],
                                scalar=1.0,
                                in1=b[:, 0:vsz],
                                op0=Alu.bypass,
                                op1=Alu.add,
                            )
                        if t0 + tsz > nv:
                            p0 = max(t0, nv)
                            if use_bout:
                                nc.scalar.activation(
                                    fin[:, p0:t0 + tsz],
                                    b[:, p0 - t0:tsz],
                                    Act.Copy,
                                    bias=bout_t[:, oc:oc + 1],
                                )
                            else:
                                nc.scalar.copy(
                                    fin[:, p0:t0 + tsz], b[:, p0 - t0:tsz]
                                )
                        if last:
                            dma = nc.sync.dma_start if oc % 2 == 0 \
                                else nc.scalar.dma_start
                            dma(
                                out=out_p[item, oc * 128:(oc + 1) * 128, :],
                                in_=fin,
                            )
                    return emit

                for oc in range(FC):
                    chunks = _n_chunks(T)
                    for i, (t0, tsz) in enumerate(chunks):
                        gs.append(make_group(oc, t0, tsz, i == len(chunks) - 1))
                return gs

            def emit_attention(item, interleave, per_slot):
                """head-pipelined attention; drains `interleave` closures
                inside the slots (per_slot per head slot).

                Slot h: dn/rec/bcs of head h-1 first (their inputs finished
                last slot, so the rec chain beats this slot's exps into the
                ScalarE queue), then scores/exp/es of head h with the
                previous head's ctx+fsmn and interleave work spread in."""
                pend = list(interleave)

                def mk_drain(budget):
                    state = [budget]

                    def drain(k):
                        n = min(k, state[0], len(pend))
                        state[0] -= n
                        for _ in range(n):
                            pend.pop(0)()
                    return drain

                prev = None  # (h, ets, es)
                for step in range(H + 1):
                    drain = mk_drain(per_slot if step < H else len(pend))
                    if step < H:
                        h = step
                        ets = []
                        es = None
                        rec_r = None
                        done_dn = done_bcs = done_ctx = prev is None
                        for kt in range(nvt):
                            ets.append(emit_scores_kt(item, h, kt))
                            es = emit_es_step(ets, es, kt)
                            if not done_dn:
                                emit_dn(prev[2])
                                rec_r = emit_rec()
                                done_dn = True
                            elif kt >= 2 and not done_bcs:
                                bcs = emit_bcs(rec_r)
                                done_bcs = True
                            elif kt >= 3 and not done_ctx:
                                emit_ctx(item, prev[0], prev[1])
                                done_ctx = True
                            drain(1)
                        if es is None:
                            es = ets[0]
                        if prev is not None:
                            if not done_bcs:
                                bcs = emit_bcs(rec_r)
                            if not done_ctx:
                                emit_ctx(item, prev[0], prev[1])
                            emit_norm_fsmn(item, prev[0], bcs)
                        drain(per_slot)
                        prev = (h, ets, es)
                    else:
                        emit_dn(prev[2])
                        rec_r = emit_rec()
                        bcs = emit_bcs(rec_r)
                        emit_ctx(item, prev[0], prev[1])
                        emit_norm_fsmn(item, prev[0], bcs)
                        drain(len(pend))
                return pend

            # =========================== schedule ===========================
            # P0: item0 projections (PE-dense; copies on ACT which is idle)
            emit_proj_waves(0, act_copy)
            # A0: item0 attention with item1 projection groups interleaved
            rest = emit_attention(0, proj_groups(1, dve_copy), per_slot=8)
            for g in rest:
                g()
            # A1: item1 attention with item0 out-projection interleaved
            rest = emit_attention(1, outproj_groups(0), per_slot=2)
            for g in rest:
                g()
            # O1: item1 out-projection
            for g in outproj_groups(1):
                g()

    _split_multiwaits(nc)
    return nc


_cache = {}


def _get_nc(nv, use_bqkv, use_bout):
    key = (nv, use_bqkv, use_bout)
    if key not in _cache:
        _cache[key] = _build(nv, use_bqkv, use_bout)
    return _cache[key]


def _make_wdiag(w_fsmn):
    """(128, FC, NPE, 128) fp16: diag(w[:, 4..10]) + identity (residual)."""
    wd = np.zeros((128, FC, NPE, 128), np.float16)
    idx = np.arange(128)
    for cc in range(FC):
        for j8 in range(NPE - 1):
            wd[idx, cc, j8, idx] = w_fsmn[cc * 128 + idx, NDVE + j8].astype(
                np.float16
            )
        wd[idx, cc, NPE - 1, idx] = 1.0
    return wd


def kernel(x, mask, w_qkv, b_qkv, w_out, b_out, w_fsmn):
    x = np.asarray(x, dtype=np.float32)
    mask = np.asarray(mask, dtype=np.float32)
    w_qkv = np.asarray(w_qkv, dtype=np.float32)
    b_qkv = np.asarray(b_qkv, dtype=np.float32)
    w_out = np.asarray(w_out, dtype=np.float32)
    b_out = np.asarray(b_out, dtype=np.float32)
    w_fsmn = np.asarray(w_fsmn, dtype=np.float32)

    assert x.shape == (B, T, F) and mask.shape == (B, 1, T)

    # mask must be a shared valid-prefix across the batch (as in batched ASR)
    m = mask.reshape(B, T)
    nv = int(round(float(m[0].sum())))
    expect = np.zeros(T, np.float32)
    expect[:nv] = 1.0
    if not np.all(m == expect[None, :]):
        raise NotImplementedError("kernel supports shared prefix masks only")
    nv = max(128, min(T, nv))

    use_bqkv = bool(np.any(b_qkv))
    use_bout = bool(np.any(b_out))
    nc = _get_nc(nv, use_bqkv, use_bout)

    nvt = _ceil_div(nv, 128)
    wdiag = _make_wdiag(w_fsmn)
    wfsmn_t = np.ascontiguousarray(
        w_fsmn.reshape(FC, 128, KERNEL).transpose(1, 0, 2)
    )
    # xT as [NB, 128, FC, T]: xT[i, p, c, t] = x[i, t, c*128+p]
    x16 = x.astype(np.float16)
    xT16 = [
        np.ascontiguousarray(
            x16[c * NB:(c + 1) * NB]
            .transpose(0, 2, 1)                     # (NB, F, T)
            .reshape(NB, FC, 128, T)
            .transpose(0, 2, 1, 3)                  # (NB, 128, FC, T)
        )
        for c in range(N_CORES)
    ]
    # wq as [128, FC, 3F]: wq[p, c, o] = w_qkv[c*128+p, o]
    wq16 = np.ascontiguousarray(
        w_qkv.astype(np.float16).reshape(FC, 128, 3 * F).transpose(1, 0, 2)
    )
    # wout as [128, FC, F]
    wout16 = np.ascontiguousarray(
        w_out.astype(np.float16).reshape(FC, 128, F).transpose(1, 0, 2)
    )
    in_maps = []
    for c in range(N_CORES):
        im = {
            "xT": xT16[c],
            "wq": wq16,
            "wout": wout16,
            "wdiag": wdiag,
            "wfsmn": wfsmn_t,
        }
        if use_bqkv:
            im["bqkv"] = np.ascontiguousarray(b_qkv[None, :])
        if use_bout:
            im["bout"] = np.ascontiguousarray(b_out.reshape(FC, 128).T)
        if nv - (nvt - 1) * 128 != 128:
            vcol = np.zeros((128, 1), np.float32)
            vcol[: nv - (nvt - 1) * 128] = 1.0
            im["vcol"] = vcol
        in_maps.append(im)

    global _last_in_maps
    _last_in_maps = in_maps
    res = run_bass_kernel_spmd(nc, in_maps, list(range(N_CORES)))
    out = np.empty((B, T, F), np.float32)
    for c in range(N_CORES):
        oT = res.results[c]["outT"]  # (NB, F, T) fp16
        for i in range(NB):
            out[c * NB + i] = oT[i].astype(np.float32).T
    return out
